# revision 43
# baseline (speedup 1.0000x reference)
import sys

sys.path.insert(0, "/opt/trn_rl_repo")

import numpy as np

NCORES = 8
B, FULL_N, D = 4, 2048, 1024
NH = 16
DK = 64  # head dim
HPC = NH // NCORES  # heads per core = 2
CW = HPC * DK  # output columns per core = 128
DC = D // 128  # D chunks = 8
VW = 80  # padded V width per head (64 dims + ones col + pad to %16)

# exp engine routing: score chunks with (2j+h) % 16 in ACT_PHASE go through
# ScalarE's exact Exp (fp8e4 out, scaled 2^-3); the rest through DVE's
# Schraudolph affine bit-hack (fp32 -> saturating uint8 == fp8e4m3 bits).
ACT_PHASE = frozenset({0, 2, 4, 6, 8, 10, 12, 14, 15})
# byte = psum_score * (8/ln2)/8 + (56 - 24 - sigma)  [fp8e4m3, y=exp(s)/8]
A_HACK = float(1.0 / np.log(2.0))  # 1.4427 (includes the 1/sqrt(dk)=1/8 fold)
B_HACK = 31.537  # 32 - 0.463 Schraudolph mantissa-balance

_CACHE = {}
LAST_RESULTS = None


def _build(n_rows):
    """SPMD Bass program for one core. Each core computes batch-0 attention
    for its 2 heads (the reference only uses att[0]) and adds it to its
    column slice of tgt for all batches.

    Structure (tuned against real-HW traces, not the cost model):
    - Q/K/V projections run fp8 DoubleRow on fp8-staged inputs; scores run
      fp16 (on this HW a matmul costs ~1 cycle per output column whatever
      the dtype/perf-mode, so fp8-DR scores buy nothing and slot-split
      projections are pure loss). V is projected directly transposed
      (memT as lhsT) so no PE transposes are needed for the PV operand.
    - Softmax exps (the 2nd bottleneck after the PE) split ScalarE (exact
      Exp, 9/16 chunks) and DVE (saturating affine-to-uint8 Schraudolph
      bit hack, 7/16); score PSUM tiles triple-buffer (sharing the pool
      with projection/finalize tiles) so neither engine idles on the
      score->exp->PV rotation.
    - PV runs fp8 DR with a ones column at position 64 of each 80-wide V
      block yielding softmax row sums for free; its accumulation pairs
      are spread one-per-odd-chunk through the score stream (clumped PV
      blocks starve the exp engines).
    - Each q-group's finalize/add/store is emitted mid-round so the
      in-order ACT/DVE queues never bury them behind the next group's
      exps; nothing but PV(3)+fin(3) remains after the last exp.
    - DMA: K-group feeds outrank Q feeds (scores need K first); the 2MB
      tgt/outc add-path loads ride the Pool queue behind filler memsets
      so they never steal inbound bandwidth from the critical loads.
    - Final adds ride GpSimd except the last q-group (split DVE/GpSimd
      to shorten the serial tail)."""
    import concourse.mybir as mybir
    import concourse.tile as tile
    from concourse import bacc
    from concourse.masks import make_identity

    fp32 = mybir.dt.float32
    fp16 = mybir.dt.float16
    bf16 = mybir.dt.bfloat16
    fp8 = mybir.dt.float8e4
    u8 = mybir.dt.uint8

    RT = n_rows // 128  # row tiles = 16
    G = n_rows // 512  # 512-row groups = 4
    QG = G
    KC = RT  # key chunks of 128
    JP = KC // 2  # key chunk pairs = 8

    nc = bacc.Bacc(None, target_bir_lowering=False)
    tgt0t = nc.declare_dram_parameter("tgt0t", [D, n_rows], fp8, isOutput=False)
    mem0t = nc.declare_dram_parameter("mem0t", [D, n_rows], fp8, isOutput=False)
    wqt = nc.declare_dram_parameter("wqt", [D, CW], fp8, isOutput=False)
    wkt = nc.declare_dram_parameter("wkt", [D, CW], fp8, isOutput=False)
    wvt = nc.declare_dram_parameter("wvt", [D, CW], fp8, isOutput=False)
    tgtc = nc.declare_dram_parameter("tgtc", [B, n_rows, CW], fp16, isOutput=False)
    outc = nc.declare_dram_parameter("outc", [B, n_rows, CW], fp16, isOutput=True)

    Exp = mybir.ActivationFunctionType.Exp
    DR = mybir.MatmulPerfMode.DoubleRow
    mult = mybir.AluOpType.mult
    add = mybir.AluOpType.add

    with tile.TileContext(nc) as tc:
        with (
            tc.tile_pool(name="const", bufs=1) as const,
            tc.tile_pool(name="persist", bufs=1) as persist,
        ):
            ident16 = const.tile([128, 128], fp16)
            make_identity(nc, ident16)
            bias_exp = const.tile([128, 1], fp32, tag="bias")
            nc.vector.memset(bias_exp, -3.0 * float(np.log(2.0)))
            wz = const.tile([128, 128], fp8, tag="wz")
            nc.vector.memset(wz, 0.0)

            # Q/K fp16 stores: partition = dim h*64+d, free = query/key
            QT = persist.tile([128, n_rows], fp16, tag="qt")
            KT = persist.tile([128, n_rows], fp16, tag="kt")
            # [keys, pair jp, pair half i, h*VW + dim]; ones at h*VW+64
            Vp = persist.tile([128, JP, 2, HPC * VW], fp8, tag="Vp")
            nc.gpsimd.memset(Vp, 0.0)
            Vp_h = Vp.rearrange("p jp i (h c) -> p jp i h c", h=HPC)
            nc.gpsimd.memset(Vp_h[:, :, :, :, DK : DK + 1], 1.0)

            att_sb = persist.tile([128, RT, CW], fp16, tag="att")
            tgtc_sb = persist.tile([128, B, RT, CW], fp16, tag="tgtc")

            with (
                tc.tile_pool(name="wst", bufs=1) as wst_pool,
                tc.tile_pool(name="memg", bufs=2) as mem_pool,
                tc.tile_pool(name="tgtg", bufs=2) as tgt_pool,
                tc.tile_pool(name="usb", bufs=2) as usb_pool,
                tc.tile_pool(name="small", bufs=4) as small_pool,
                tc.tile_pool(name="pt", bufs=2) as pt_pool,
                tc.tile_pool(name="ps_st", bufs=3, space="PSUM") as ps_st,
                tc.tile_pool(name="ps_u", bufs=2, space="PSUM") as ps_u,
            ):
                # --- initial DMAs ---
                # ACT queue: mem0 first (K0 gates the first scores), then the
                # Exp table, then mem2 (key groups keep landing while SP
                # drains tgt0/mem1). SP: weights, tgt0, mem1, tgt1, mem3, ...
                wt_q = wst_pool.tile([128, DC, CW], fp8, tag="wtq")
                wt_k = wst_pool.tile([128, DC, CW], fp8, tag="wtk")
                wt_v = wst_pool.tile([128, DC, CW], fp8, tag="wtv")
                WTs = {"q": wt_q, "k": wt_k, "v": wt_v}

                mem_tiles = {}
                tgt_tiles = {}

                def emit_mem_dma(g, eng):
                    t = mem_pool.tile([128, DC, 512], fp8, tag="memg", name=f"mem{g}")
                    eng.dma_start(
                        out=t,
                        in_=mem0t[:, g * 512 : (g + 1) * 512].rearrange(
                            "(c p) n -> p c n", p=128
                        ),
                    )
                    mem_tiles[g] = t

                def emit_tgt_dma(g, eng):
                    t = tgt_pool.tile([128, DC, 512], fp8, tag="tgtg", name=f"tgt{g}")
                    eng.dma_start(
                        out=t,
                        in_=tgt0t[:, g * 512 : (g + 1) * 512].rearrange(
                            "(c p) n -> p c n", p=128
                        ),
                    )
                    tgt_tiles[g] = t

                nc.sync.dma_start(
                    out=wt_q, in_=wqt[:, :].rearrange("(c p) q -> p c q", p=128)
                )
                emit_tgt_dma(0, nc.sync)
                nc.sync.dma_start(
                    out=wt_k, in_=wkt[:, :].rearrange("(c p) q -> p c q", p=128)
                )
                emit_mem_dma(0, nc.sync)
                # table preload for the exact-Exp chunks
                act_warm = const.tile([128, 1], fp32, tag="actw")
                nc.scalar.activation(out=act_warm, in_=bias_exp, func=Exp)
                emit_mem_dma(2, nc.scalar)
                nc.sync.dma_start(
                    out=wt_v, in_=wvt[:, :].rearrange("(c p) q -> p c q", p=128)
                )
                emit_tgt_dma(1, nc.sync)
                emit_mem_dma(1, nc.sync)
                emit_mem_dma(3, nc.sync)
                emit_tgt_dma(2, nc.sync)
                emit_tgt_dma(3, nc.sync)
                # tgtc loads ride the Pool queue BEHIND scratch-memset
                # fillers so their 2MB doesn't steal inbound DMA bandwidth
                # from the critical startup loads (Pool executes in order;
                # the adds that consume tgtc_sb only start much later).
                scratch = persist.tile([128, 4096], fp16, tag="scratch")
                for r in range(4):
                    nc.gpsimd.memset(scratch, 0.0)
                for b in range(B):
                    nc.gpsimd.dma_start(
                        out=tgtc_sb[:, b, 0:8, :],
                        in_=tgtc[b, 0:1024, :].rearrange("(t p) c -> p t c", p=128),
                    )
                for r in range(3):
                    nc.gpsimd.memset(scratch, 0.0)
                for b in range(B):
                    nc.gpsimd.dma_start(
                        out=tgtc_sb[:, b, 8:16, :],
                        in_=tgtc[b, 1024:2048, :].rearrange("(t p) c -> p t c", p=128),
                    )

                def alloc_qk(name):
                    # proj/vps/ta tiles live in the st pool (uniform shape;
                    # every consumer completes independently, so sharing is
                    # deadlock-free and buys a 3rd score buffer)
                    return ps_st.tile([128, 2, 512], fp32, tag="st", name=name)

                def emit_k_proj(pk, g):
                    src = mem_tiles[g]
                    for dp in range(DC // 2):
                        nc.tensor.matmul(
                            pk[:, 0, :],
                            WTs["k"][:, 2 * dp : 2 * dp + 2, :],
                            src[:, 2 * dp : 2 * dp + 2, :],
                            start=(dp == 0), stop=(dp == DC // 2 - 1),
                            perf_mode=DR,
                        )

                def emit_q_proj(pk, g):
                    src = tgt_tiles[g]
                    for dp in range(DC // 2):
                        nc.tensor.matmul(
                            pk[:, 0, :],
                            WTs["q"][:, 2 * dp : 2 * dp + 2, :],
                            src[:, 2 * dp : 2 * dp + 2, :],
                            start=(dp == 0), stop=(dp == DC // 2 - 1),
                            perf_mode=DR,
                        )

                def emit_qk_cast(pk, g, eng, part):
                    dst = (KT if part == "k" else QT)[:, g * 512 : (g + 1) * 512]
                    src = pk[:, 0, :]
                    if eng == "act":
                        nc.scalar.copy(out=dst, in_=src)
                    else:
                        nc.vector.tensor_copy(out=dst, in_=src)

                def alloc_u(name):
                    return ps_u.tile([128, 512], fp32, tag="u", name=name)

                def emit_vt(g):
                    """V^T for group g: [128 keys, 4 chunks, 128 dims] via
                    DR matmuls with memT as lhsT (no transposes needed).
                    Lives in the u pool so its cast never blocks the
                    exp-paced score-tile rotation."""
                    vt = alloc_u(f"vps{g}")
                    vps = vt.rearrange("p (t c) -> p t c", t=4)
                    src = mem_tiles[g]
                    for t in range(4):
                        for dp in range(DC // 2):
                            nc.tensor.matmul(
                                vps[:, t, :],
                                src[:, 2 * dp : 2 * dp + 2, t * 128 : (t + 1) * 128],
                                WTs["v"][:, 2 * dp : 2 * dp + 2, :],
                                start=(dp == 0), stop=(dp == DC // 2 - 1),
                                perf_mode=DR,
                            )
                    return vps

                def emit_v_cast(g, vps, eng):
                    # [128, (jp2 i2), (h2, 64)] -> Vp[:, 2g:2g+2, :, :, 0:64]
                    src = vps.rearrange("p (jp i) (h c) -> p jp i h c", jp=2, h=HPC)
                    dst = Vp_h[:, 2 * g : 2 * g + 2, :, :, 0:DK]
                    if eng == "act":
                        nc.scalar.copy(out=dst, in_=src)
                    else:
                        nc.vector.tensor_copy(out=dst, in_=src)

                def emit_score_chunk(qg, j, pts):
                    """Score matmuls (one per head; each out [128,512] fits a
                    single PSUM bank) + one both-heads exp for key chunk j."""
                    st = ps_st.tile(
                        [128, HPC, 512], fp32, tag="st", name=f"st{qg}_{j}"
                    )
                    for h in range(HPC):
                        hs = h * DK
                        nc.tensor.matmul(
                            st[:, h, :],
                            KT[hs : hs + DK, j * 128 : (j + 1) * 128],
                            QT[hs : hs + DK, qg * 512 : (qg + 1) * 512],
                            start=True, stop=True,
                        )
                    jp, i = j // 2, j % 2
                    dst = pts[:, jp, i, :, :]  # [128, h, 512] fp8
                    if j in ACT_PHASE:
                        nc.scalar.activation(
                            out=dst, in_=st, func=Exp, scale=0.125, bias=bias_exp
                        )
                    else:
                        nc.vector.tensor_scalar(
                            out=dst.bitcast(u8), in0=st,
                            scalar1=A_HACK, scalar2=B_HACK,
                            op0=mult, op1=add,
                        )

                def emit_pv_pair(pu_list, jp, pts):
                    for h in range(HPC):
                        nc.tensor.matmul(
                            pu_list[h][0:VW, :],
                            Vp[:, jp, :, h * VW : (h + 1) * VW],
                            pts[:, jp, :, h, :],
                            start=(jp == 0), stop=(jp == JP - 1),
                            perf_mode=DR,
                        )

                def emit_pv(qg, pu_list, pts):
                    for jp in range(JP):
                        emit_pv_pair(pu_list, jp, pts)

                def emit_finalize(qg, pu_list):
                    """pu [80,512] PSUM -> fp16 SBUF casts (h0 DVE, h1 ACT),
                    8 PE transposes into a recycled qk-pool tile (bitcast to
                    fp16), one batched reciprocal of the ones-column sums,
                    then 2x-mode scales writing att_sb."""
                    pu_sbs = []
                    for h in range(HPC):
                        pu_sb = usb_pool.tile([VW, 512], fp16, tag="usb")
                        if h == 1:
                            nc.scalar.copy(out=pu_sb, in_=pu_list[h][0:VW, :])
                        else:
                            nc.vector.tensor_copy(out=pu_sb, in_=pu_list[h][0:VW, :])
                        pu_sbs.append(pu_sb)
                    ta_raw = alloc_qk(f"ta{qg}")
                    # fp16 view: [128, 2h, 4s, 256] (only 0:80 of each used)
                    ta = ta_raw.bitcast(fp16).rearrange(
                        "p h (s c) -> p h s c", s=4
                    )
                    for h in range(HPC):
                        for s in range(4):
                            nc.tensor.transpose(
                                ta[:, h, s, 0:VW],
                                pu_sbs[h][:, s * 128 : (s + 1) * 128],
                                ident16[0:VW, 0:VW],
                            )
                    rec = small_pool.tile([128, 2, 4, 1], fp32, tag="rec")
                    nc.vector.reciprocal(rec, ta[:, :, :, DK : DK + 1])
                    for h in range(HPC):
                        nc.vector.tensor_tensor(
                            out=att_sb[:, qg * 4 : (qg + 1) * 4, h * DK : (h + 1) * DK],
                            in0=ta[:, h, :, 0:DK],
                            in1=rec[:, h, :, :].to_broadcast((128, 4, DK)),
                            op=mult,
                        )

                def emit_add_store(qg, last):
                    qsl = slice(qg * 512, (qg + 1) * 512)
                    for b in range(B):
                        eng = nc.vector if (last and b % 2 == 0) else nc.gpsimd
                        eng.tensor_add(
                            out=tgtc_sb[:, b, qg * 4 : (qg + 1) * 4, :],
                            in0=tgtc_sb[:, b, qg * 4 : (qg + 1) * 4, :],
                            in1=att_sb[:, qg * 4 : (qg + 1) * 4, :],
                        )
                        eng2 = nc.scalar if (last and b >= 2) else nc.sync
                        eng2.dma_start(
                            out=outc[b, qsl, :].rearrange("(t p) c -> p t c", p=128),
                            in_=tgtc_sb[:, b, qg * 4 : (qg + 1) * 4, :],
                        )

                # ---- Phase A: projections, V, qg0+qg1 scores ----
                pts_tiles = {}
                pus = {}
                pts_tiles[0] = pt_pool.tile(
                    [128, JP, 2, HPC, 512], fp8, tag="pts", name="pts0"
                )
                pts_tiles[1] = pt_pool.tile(
                    [128, JP, 2, HPC, 512], fp8, tag="pts", name="pts1"
                )

                pq0 = alloc_qk("pq0")
                for _ in range(12):
                    nc.tensor.matmul(
                        pq0[:, 0, 0:128], wz, wz,
                        start=True, stop=True, skip_group_check=True,
                    )
                emit_q_proj(pq0, 0)
                emit_qk_cast(pq0, 0, "dve", part="q")
                pk0 = alloc_qk("pk0")
                emit_k_proj(pk0, 0)
                emit_qk_cast(pk0, 0, "act", part="k")
                for j in range(0, 4):
                    emit_score_chunk(0, j, pts_tiles[0])
                vps0 = emit_vt(0)
                emit_v_cast(0, vps0, "dve")
                # g2 landed early on the ACT queue
                pk2 = alloc_qk("pk2")
                emit_k_proj(pk2, 2)
                emit_qk_cast(pk2, 2, "act", part="k")
                for j in range(8, 12):
                    emit_score_chunk(0, j, pts_tiles[0])
                vps2 = emit_vt(2)
                emit_v_cast(2, vps2, "act")
                pq1 = alloc_qk("pq1")
                emit_q_proj(pq1, 1)
                emit_qk_cast(pq1, 1, "dve", part="q")
                pk1 = alloc_qk("pk1")
                emit_k_proj(pk1, 1)
                emit_qk_cast(pk1, 1, "act", part="k")
                for j in range(4, 8):
                    emit_score_chunk(0, j, pts_tiles[0])
                vps1 = emit_vt(1)
                emit_v_cast(1, vps1, "dve")
                pk3 = alloc_qk("pk3")
                emit_k_proj(pk3, 3)
                emit_qk_cast(pk3, 3, "dve", part="k")
                for j in range(12, 16):
                    emit_score_chunk(0, j, pts_tiles[0])
                vps3 = emit_vt(3)
                emit_v_cast(3, vps3, "act")
                pus[0] = [alloc_u(f"u0_{h}") for h in range(HPC)]
                for jp in range(4):
                    emit_pv_pair(pus[0], jp, pts_tiles[0])

                # ---- Phase B ----
                # Strict qg-major chunk order; each PV chain's pairs are
                # interleaved two-at-a-time into the first half of the NEXT
                # q-group's score stream (deps are ~16 chunks stale), and
                # fin/add/store fire immediately after the chain closes so
                # nothing cascades into the tail.
                def emit_round(g_sc, qg_pv, pts_sc):
                    # first half: scores(g_sc, 0..7) with PV(qg_pv) pairs
                    # 4..7 (one per odd chunk); fin/add/store(qg_pv); second
                    # half: scores 8..15 with PV(g_sc-1) pairs 0..3 --
                    # spreading PV evenly keeps the PE from starving the
                    # exp stream in bursts.
                    for j in range(0, 8):
                        emit_score_chunk(g_sc, j, pts_sc)
                        if j % 2 == 1:
                            emit_pv_pair(pus[qg_pv], 4 + j // 2, pts_tiles[qg_pv])
                    emit_finalize(qg_pv, pus[qg_pv])
                    emit_add_store(qg_pv, last=(qg_pv == QG - 1))
                    qg_n = g_sc
                    pus[qg_n] = [alloc_u(f"u{qg_n}_{h}") for h in range(HPC)]
                    for j in range(8, 16):
                        emit_score_chunk(g_sc, j, pts_sc)
                        if j % 2 == 1:
                            emit_pv_pair(pus[qg_n], (j - 9) // 2, pts_tiles[qg_n])

                # qg1 scores with PV(0) interleaved
                emit_round(1, 0, pts_tiles[1])
                # qg2 scores with PV(1)
                pq2 = alloc_qk("pq2")
                emit_q_proj(pq2, 2)
                emit_qk_cast(pq2, 2, "act", part="q")
                pts_tiles[2] = pt_pool.tile(
                    [128, JP, 2, HPC, 512], fp8, tag="pts", name="pts2"
                )
                emit_round(2, 1, pts_tiles[2])
                # qg3 scores with PV(2)
                pq3 = alloc_qk("pq3")
                emit_q_proj(pq3, 3)
                emit_qk_cast(pq3, 3, "dve", part="q")
                pts_tiles[3] = pt_pool.tile(
                    [128, JP, 2, HPC, 512], fp8, tag="pts", name="pts3"
                )
                emit_round(3, 2, pts_tiles[3])
                # PV(3) pairs 4..7 trickle against the last exps
                for jp in range(4, JP):
                    emit_pv_pair(pus[3], jp, pts_tiles[3])
                emit_finalize(3, pus[3])
                emit_add_store(3, last=True)

    nc.finalize()
    return nc


def _get_nc(n_rows):
    if n_rows not in _CACHE:
        _CACHE[n_rows] = _build(n_rows)
    return _CACHE[n_rows]


def _run(tgt, memory, Wq, Wk, Wv, trace=False):
    global LAST_RESULTS
    from concourse.bass_utils import run_bass_kernel_spmd

    n_rows = tgt.shape[1]
    nc = _get_nc(n_rows)

    tgt = np.ascontiguousarray(tgt, dtype=np.float32)
    memory = np.ascontiguousarray(memory, dtype=np.float32)
    import ml_dtypes

    f8 = ml_dtypes.float8_e4m3
    tgt0t = np.ascontiguousarray(tgt[0].T).astype(f8)
    mem0t = np.ascontiguousarray(memory[0].T).astype(f8)

    in_maps = []
    for c in range(NCORES):
        sl = slice(c * CW, (c + 1) * CW)
        in_maps.append(
            {
                "tgt0t": tgt0t,
                "mem0t": mem0t,
                "wqt": np.ascontiguousarray(Wq[sl, :].T).astype(f8),
                "wkt": np.ascontiguousarray(Wk[sl, :].T).astype(f8),
                "wvt": np.ascontiguousarray(Wv[sl, :].T).astype(f8),
                "tgtc": np.ascontiguousarray(tgt[:, :, sl]).astype(np.float16),
            }
        )
    res = run_bass_kernel_spmd(nc, in_maps, list(range(NCORES)), trace=trace)
    LAST_RESULTS = res
    out = np.concatenate(
        [res.results[c]["outc"].astype(np.float32) for c in range(NCORES)], axis=2
    )
    return out


def kernel(tgt, memory, Wq, Wk, Wv):
    return _run(tgt, memory, Wq, Wk, Wv)


# revision 44
# speedup vs baseline: 1.1231x; 1.1231x over previous
import sys

sys.path.insert(0, "/opt/trn_rl_repo")

import numpy as np

NCORES = 8
B, FULL_N, D = 4, 2048, 1024
NH = 16
DK = 64  # head dim
HPC = NH // NCORES  # heads per core = 2
CW = HPC * DK  # output columns per core = 128
DC = D // 128  # D chunks = 8
VW = 80  # padded V width per head (64 dims + ones col + pad to %16)

# exp engine routing: score chunks with (2j+h) % 16 in ACT_PHASE go through
# ScalarE's exact Exp (fp8e4 out, scaled 2^-3); the rest through DVE's
# Schraudolph affine bit-hack (fp32 -> saturating uint8 == fp8e4m3 bits).
ACT_PHASE = frozenset({0, 2, 4, 6, 8, 10, 12, 14, 15})
# byte = psum_score * (8/ln2)/8 + (56 - 24 - sigma)  [fp8e4m3, y=exp(s)/8]
A_HACK = float(1.0 / np.log(2.0))  # 1.4427 (includes the 1/sqrt(dk)=1/8 fold)
B_HACK = 31.537  # 32 - 0.463 Schraudolph mantissa-balance

_CACHE = {}
LAST_RESULTS = None


def _build(n_rows):
    """SPMD Bass program for one core. Each core computes batch-0 attention
    for its 2 heads (the reference only uses att[0]) and adds it to its
    column slice of tgt for all batches.

    Structure (tuned against real-HW traces, not the cost model):
    - Q/K/V projections run fp8 DoubleRow on fp8-staged inputs; scores run
      fp16 (on this HW a matmul costs ~1 cycle per output column whatever
      the dtype/perf-mode, so fp8-DR scores buy nothing and slot-split
      projections are pure loss). V is projected directly transposed
      (memT as lhsT) so no PE transposes are needed for the PV operand.
    - Softmax exps (the 2nd bottleneck after the PE) split ScalarE (exact
      Exp, 9/16 chunks) and DVE (saturating affine-to-uint8 Schraudolph
      bit hack, 7/16); score PSUM tiles triple-buffer (sharing the pool
      with projection/finalize tiles) so neither engine idles on the
      score->exp->PV rotation.
    - PV runs fp8 DR with a ones column at position 64 of each 80-wide V
      block yielding softmax row sums for free; its accumulation pairs
      are spread one-per-odd-chunk through the score stream (clumped PV
      blocks starve the exp engines).
    - Each q-group's finalize/add/store is emitted mid-round so the
      in-order ACT/DVE queues never bury them behind the next group's
      exps; nothing but PV(3)+fin(3) remains after the last exp.
    - DMA: K-group feeds outrank Q feeds (scores need K first); the 2MB
      tgt/outc add-path loads ride the Pool queue behind filler memsets
      so they never steal inbound bandwidth from the critical loads.
    - Final adds ride GpSimd except the last q-group (split DVE/GpSimd
      to shorten the serial tail)."""
    import concourse.mybir as mybir
    import concourse.tile as tile
    from concourse import bacc
    from concourse.masks import make_identity

    fp32 = mybir.dt.float32
    fp16 = mybir.dt.float16
    bf16 = mybir.dt.bfloat16
    fp8 = mybir.dt.float8e4
    u8 = mybir.dt.uint8

    RT = n_rows // 128  # row tiles = 16
    G = n_rows // 512  # 512-row groups = 4
    QG = G
    KC = RT  # key chunks of 128
    JP = KC // 2  # key chunk pairs = 8

    nc = bacc.Bacc(None, target_bir_lowering=False)
    tgt0t = nc.declare_dram_parameter("tgt0t", [D, n_rows], fp8, isOutput=False)
    mem0t = nc.declare_dram_parameter("mem0t", [D, n_rows], fp8, isOutput=False)
    wqt = nc.declare_dram_parameter("wqt", [D, CW], fp8, isOutput=False)
    wkt = nc.declare_dram_parameter("wkt", [D, CW], fp8, isOutput=False)
    wvt = nc.declare_dram_parameter("wvt", [D, CW], fp8, isOutput=False)
    tgtc = nc.declare_dram_parameter("tgtc", [B, n_rows, CW], fp16, isOutput=False)
    outc = nc.declare_dram_parameter("outc", [B, n_rows, CW], fp16, isOutput=True)

    Exp = mybir.ActivationFunctionType.Exp
    DR = mybir.MatmulPerfMode.DoubleRow
    mult = mybir.AluOpType.mult
    add = mybir.AluOpType.add

    with tile.TileContext(nc) as tc:
        with (
            tc.tile_pool(name="const", bufs=1) as const,
            tc.tile_pool(name="persist", bufs=1) as persist,
        ):
            ident16 = const.tile([128, 128], fp16)
            make_identity(nc, ident16)
            bias_exp = const.tile([128, 1], fp32, tag="bias")
            nc.vector.memset(bias_exp, -3.0 * float(np.log(2.0)))
            wz = const.tile([128, 128], fp8, tag="wz")
            nc.vector.memset(wz, 0.0)

            # Q/K fp16 stores: partition = dim h*64+d, free = query/key
            QT = persist.tile([128, n_rows], fp16, tag="qt")
            KT = persist.tile([128, n_rows], fp16, tag="kt")
            # [keys, pair jp, pair half i, h*VW + dim]; ones at h*VW+64
            Vp = persist.tile([128, JP, 2, HPC * VW], fp8, tag="Vp")
            nc.gpsimd.memset(Vp, 0.0)
            Vp_h = Vp.rearrange("p jp i (h c) -> p jp i h c", h=HPC)
            nc.gpsimd.memset(Vp_h[:, :, :, :, DK : DK + 1], 1.0)

            att_sb = persist.tile([128, RT, CW], fp16, tag="att")
            tgtc_sb = persist.tile([128, B, RT, CW], fp16, tag="tgtc")

            with (
                tc.tile_pool(name="wst", bufs=1) as wst_pool,
                tc.tile_pool(name="memg", bufs=2) as mem_pool,
                tc.tile_pool(name="tgtg", bufs=2) as tgt_pool,
                tc.tile_pool(name="usb", bufs=2) as usb_pool,
                tc.tile_pool(name="small", bufs=4) as small_pool,
                tc.tile_pool(name="pt", bufs=2) as pt_pool,
                tc.tile_pool(name="ps_st", bufs=3, space="PSUM") as ps_st,
                tc.tile_pool(name="ps_u", bufs=2, space="PSUM") as ps_u,
            ):
                # --- initial DMAs ---
                # ACT queue: mem0 first (K0 gates the first scores), then the
                # Exp table, then mem2 (key groups keep landing while SP
                # drains tgt0/mem1). SP: weights, tgt0, mem1, tgt1, mem3, ...
                wt_q = wst_pool.tile([128, DC, CW], fp8, tag="wtq")
                wt_k = wst_pool.tile([128, DC, CW], fp8, tag="wtk")
                wt_v = wst_pool.tile([128, DC, CW], fp8, tag="wtv")
                WTs = {"q": wt_q, "k": wt_k, "v": wt_v}

                mem_tiles = {}
                tgt_tiles = {}

                def emit_mem_dma(g, eng):
                    t = mem_pool.tile([128, DC, 512], fp8, tag="memg", name=f"mem{g}")
                    eng.dma_start(
                        out=t,
                        in_=mem0t[:, g * 512 : (g + 1) * 512].rearrange(
                            "(c p) n -> p c n", p=128
                        ),
                    )
                    mem_tiles[g] = t

                def emit_tgt_dma(g, eng):
                    t = tgt_pool.tile([128, DC, 512], fp8, tag="tgtg", name=f"tgt{g}")
                    eng.dma_start(
                        out=t,
                        in_=tgt0t[:, g * 512 : (g + 1) * 512].rearrange(
                            "(c p) n -> p c n", p=128
                        ),
                    )
                    tgt_tiles[g] = t

                nc.sync.dma_start(
                    out=wt_q, in_=wqt[:, :].rearrange("(c p) q -> p c q", p=128)
                )
                emit_tgt_dma(0, nc.sync)
                nc.sync.dma_start(
                    out=wt_k, in_=wkt[:, :].rearrange("(c p) q -> p c q", p=128)
                )
                emit_mem_dma(0, nc.sync)
                # table preload for the exact-Exp chunks
                act_warm = const.tile([128, 1], fp32, tag="actw")
                nc.scalar.activation(out=act_warm, in_=bias_exp, func=Exp)
                emit_mem_dma(2, nc.scalar)
                emit_mem_dma(1, nc.sync)
                emit_mem_dma(3, nc.sync)
                nc.sync.dma_start(
                    out=wt_v, in_=wvt[:, :].rearrange("(c p) q -> p c q", p=128)
                )
                emit_tgt_dma(1, nc.sync)
                emit_tgt_dma(2, nc.sync)
                emit_tgt_dma(3, nc.sync)
                # tgtc loads ride the Pool queue BEHIND scratch-memset
                # fillers so their 2MB doesn't steal inbound DMA bandwidth
                # from the critical startup loads (Pool executes in order;
                # the adds that consume tgtc_sb only start much later).
                scratch = persist.tile([128, 4096], fp16, tag="scratch")
                for r in range(4):
                    nc.gpsimd.memset(scratch, 0.0)
                for b in range(B):
                    nc.gpsimd.dma_start(
                        out=tgtc_sb[:, b, 0:8, :],
                        in_=tgtc[b, 0:1024, :].rearrange("(t p) c -> p t c", p=128),
                    )
                for r in range(3):
                    nc.gpsimd.memset(scratch, 0.0)
                for b in range(B):
                    nc.gpsimd.dma_start(
                        out=tgtc_sb[:, b, 8:16, :],
                        in_=tgtc[b, 1024:2048, :].rearrange("(t p) c -> p t c", p=128),
                    )

                def alloc_qk(name):
                    # proj/vps/ta tiles live in the st pool (uniform shape;
                    # every consumer completes independently, so sharing is
                    # deadlock-free and buys a 3rd score buffer)
                    return ps_st.tile([128, 2, 512], fp32, tag="st", name=name)

                def emit_k_proj(pk, g):
                    src = mem_tiles[g]
                    for dp in range(DC // 2):
                        nc.tensor.matmul(
                            pk[:, 0, :],
                            WTs["k"][:, 2 * dp : 2 * dp + 2, :],
                            src[:, 2 * dp : 2 * dp + 2, :],
                            start=(dp == 0), stop=(dp == DC // 2 - 1),
                            perf_mode=DR,
                        )

                def emit_q_proj(pk, g):
                    src = tgt_tiles[g]
                    for dp in range(DC // 2):
                        nc.tensor.matmul(
                            pk[:, 0, :],
                            WTs["q"][:, 2 * dp : 2 * dp + 2, :],
                            src[:, 2 * dp : 2 * dp + 2, :],
                            start=(dp == 0), stop=(dp == DC // 2 - 1),
                            perf_mode=DR,
                        )

                def emit_qk_cast(pk, g, eng, part):
                    dst = (KT if part == "k" else QT)[:, g * 512 : (g + 1) * 512]
                    src = pk[:, 0, :]
                    if eng == "act":
                        nc.scalar.copy(out=dst, in_=src)
                    else:
                        nc.vector.tensor_copy(out=dst, in_=src)

                def alloc_u(name):
                    return ps_u.tile([128, 512], fp32, tag="u", name=name)

                def emit_vt(g):
                    """V^T for group g: [128 keys, 4 chunks, 128 dims] via
                    DR matmuls with memT as lhsT (no transposes needed).
                    Lives in the u pool so its cast never blocks the
                    exp-paced score-tile rotation."""
                    vt = alloc_u(f"vps{g}")
                    vps = vt.rearrange("p (t c) -> p t c", t=4)
                    src = mem_tiles[g]
                    for t in range(4):
                        for dp in range(DC // 2):
                            nc.tensor.matmul(
                                vps[:, t, :],
                                src[:, 2 * dp : 2 * dp + 2, t * 128 : (t + 1) * 128],
                                WTs["v"][:, 2 * dp : 2 * dp + 2, :],
                                start=(dp == 0), stop=(dp == DC // 2 - 1),
                                perf_mode=DR,
                            )
                    return vps

                def emit_v_cast(g, vps, eng):
                    # [128, (jp2 i2), (h2, 64)] -> Vp[:, 2g:2g+2, :, :, 0:64]
                    src = vps.rearrange("p (jp i) (h c) -> p jp i h c", jp=2, h=HPC)
                    dst = Vp_h[:, 2 * g : 2 * g + 2, :, :, 0:DK]
                    if eng == "act":
                        nc.scalar.copy(out=dst, in_=src)
                    else:
                        nc.vector.tensor_copy(out=dst, in_=src)

                def emit_score_chunk(qg, j, pts):
                    """Score matmuls (one per head; each out [128,512] fits a
                    single PSUM bank) + one both-heads exp for key chunk j."""
                    st = ps_st.tile(
                        [128, HPC, 512], fp32, tag="st", name=f"st{qg}_{j}"
                    )
                    for h in range(HPC):
                        hs = h * DK
                        nc.tensor.matmul(
                            st[:, h, :],
                            KT[hs : hs + DK, j * 128 : (j + 1) * 128],
                            QT[hs : hs + DK, qg * 512 : (qg + 1) * 512],
                            start=True, stop=True,
                        )
                    jp, i = j // 2, j % 2
                    dst = pts[:, jp, i, :, :]  # [128, h, 512] fp8
                    if j in ACT_PHASE:
                        nc.scalar.activation(
                            out=dst, in_=st, func=Exp, scale=0.125, bias=bias_exp
                        )
                    else:
                        nc.vector.tensor_scalar(
                            out=dst.bitcast(u8), in0=st,
                            scalar1=A_HACK, scalar2=B_HACK,
                            op0=mult, op1=add,
                        )

                def emit_pv_pair(pu_list, jp, pts):
                    for h in range(HPC):
                        nc.tensor.matmul(
                            pu_list[h][0:VW, :],
                            Vp[:, jp, :, h * VW : (h + 1) * VW],
                            pts[:, jp, :, h, :],
                            start=(jp == 0), stop=(jp == JP - 1),
                            perf_mode=DR,
                        )

                def emit_pv(qg, pu_list, pts):
                    for jp in range(JP):
                        emit_pv_pair(pu_list, jp, pts)

                def emit_finalize(qg, pu_list):
                    """pu [80,512] PSUM -> fp16 SBUF casts (h0 DVE, h1 ACT),
                    8 PE transposes into a recycled qk-pool tile (bitcast to
                    fp16), one batched reciprocal of the ones-column sums,
                    then 2x-mode scales writing att_sb."""
                    pu_sbs = []
                    for h in range(HPC):
                        pu_sb = usb_pool.tile([VW, 512], fp16, tag="usb")
                        if h == 1:
                            nc.scalar.copy(out=pu_sb, in_=pu_list[h][0:VW, :])
                        else:
                            nc.vector.tensor_copy(out=pu_sb, in_=pu_list[h][0:VW, :])
                        pu_sbs.append(pu_sb)
                    ta_raw = alloc_qk(f"ta{qg}")
                    # fp16 view: [128, 2h, 4s, 256] (only 0:80 of each used)
                    ta = ta_raw.bitcast(fp16).rearrange(
                        "p h (s c) -> p h s c", s=4
                    )
                    for h in range(HPC):
                        for s in range(4):
                            nc.tensor.transpose(
                                ta[:, h, s, 0:VW],
                                pu_sbs[h][:, s * 128 : (s + 1) * 128],
                                ident16[0:VW, 0:VW],
                            )
                    rec = small_pool.tile([128, 2, 4, 1], fp32, tag="rec")
                    nc.vector.reciprocal(rec, ta[:, :, :, DK : DK + 1])
                    for h in range(HPC):
                        nc.vector.tensor_tensor(
                            out=att_sb[:, qg * 4 : (qg + 1) * 4, h * DK : (h + 1) * DK],
                            in0=ta[:, h, :, 0:DK],
                            in1=rec[:, h, :, :].to_broadcast((128, 4, DK)),
                            op=mult,
                        )

                def emit_add_store(qg, last):
                    qsl = slice(qg * 512, (qg + 1) * 512)
                    for b in range(B):
                        eng = nc.vector if (last and b % 2 == 0) else nc.gpsimd
                        eng.tensor_add(
                            out=tgtc_sb[:, b, qg * 4 : (qg + 1) * 4, :],
                            in0=tgtc_sb[:, b, qg * 4 : (qg + 1) * 4, :],
                            in1=att_sb[:, qg * 4 : (qg + 1) * 4, :],
                        )
                        eng2 = nc.scalar if (last and b >= 2) else nc.sync
                        eng2.dma_start(
                            out=outc[b, qsl, :].rearrange("(t p) c -> p t c", p=128),
                            in_=tgtc_sb[:, b, qg * 4 : (qg + 1) * 4, :],
                        )

                # ---- Phase A: projections, V, qg0+qg1 scores ----
                pts_tiles = {}
                pus = {}
                pts_tiles[0] = pt_pool.tile(
                    [128, JP, 2, HPC, 512], fp8, tag="pts", name="pts0"
                )
                pts_tiles[1] = pt_pool.tile(
                    [128, JP, 2, HPC, 512], fp8, tag="pts", name="pts1"
                )

                pq0 = alloc_qk("pq0")
                for _ in range(12):
                    nc.tensor.matmul(
                        pq0[:, 0, 0:128], wz, wz,
                        start=True, stop=True, skip_group_check=True,
                    )
                emit_q_proj(pq0, 0)
                emit_qk_cast(pq0, 0, "dve", part="q")
                pk0 = alloc_qk("pk0")
                emit_k_proj(pk0, 0)
                emit_qk_cast(pk0, 0, "act", part="k")
                for j in range(0, 4):
                    emit_score_chunk(0, j, pts_tiles[0])
                vps0 = emit_vt(0)
                emit_v_cast(0, vps0, "dve")
                # g2 landed early on the ACT queue
                pk2 = alloc_qk("pk2")
                emit_k_proj(pk2, 2)
                emit_qk_cast(pk2, 2, "act", part="k")
                for j in range(8, 12):
                    emit_score_chunk(0, j, pts_tiles[0])
                vps2 = emit_vt(2)
                emit_v_cast(2, vps2, "act")
                pq1 = alloc_qk("pq1")
                emit_q_proj(pq1, 1)
                emit_qk_cast(pq1, 1, "dve", part="q")
                pk1 = alloc_qk("pk1")
                emit_k_proj(pk1, 1)
                emit_qk_cast(pk1, 1, "act", part="k")
                for j in range(4, 8):
                    emit_score_chunk(0, j, pts_tiles[0])
                vps1 = emit_vt(1)
                emit_v_cast(1, vps1, "dve")
                pk3 = alloc_qk("pk3")
                emit_k_proj(pk3, 3)
                emit_qk_cast(pk3, 3, "dve", part="k")
                for j in range(12, 16):
                    emit_score_chunk(0, j, pts_tiles[0])
                vps3 = emit_vt(3)
                emit_v_cast(3, vps3, "act")
                pus[0] = [alloc_u(f"u0_{h}") for h in range(HPC)]
                for jp in range(4):
                    emit_pv_pair(pus[0], jp, pts_tiles[0])

                # ---- Phase B ----
                # Strict qg-major chunk order; each PV chain's pairs are
                # interleaved two-at-a-time into the first half of the NEXT
                # q-group's score stream (deps are ~16 chunks stale), and
                # fin/add/store fire immediately after the chain closes so
                # nothing cascades into the tail.
                def emit_round(g_sc, qg_pv, pts_sc):
                    # first half: scores(g_sc, 0..7) with PV(qg_pv) pairs
                    # 4..7 (one per odd chunk); fin/add/store(qg_pv); second
                    # half: scores 8..15 with PV(g_sc-1) pairs 0..3 --
                    # spreading PV evenly keeps the PE from starving the
                    # exp stream in bursts.
                    for j in range(0, 8):
                        emit_score_chunk(g_sc, j, pts_sc)
                        if j % 2 == 1:
                            emit_pv_pair(pus[qg_pv], 4 + j // 2, pts_tiles[qg_pv])
                    emit_finalize(qg_pv, pus[qg_pv])
                    emit_add_store(qg_pv, last=(qg_pv == QG - 1))
                    qg_n = g_sc
                    pus[qg_n] = [alloc_u(f"u{qg_n}_{h}") for h in range(HPC)]
                    for j in range(8, 16):
                        emit_score_chunk(g_sc, j, pts_sc)
                        if j % 2 == 1:
                            emit_pv_pair(pus[qg_n], (j - 9) // 2, pts_tiles[qg_n])

                # qg1 scores with PV(0) interleaved
                emit_round(1, 0, pts_tiles[1])
                # qg2 scores with PV(1)
                pq2 = alloc_qk("pq2")
                emit_q_proj(pq2, 2)
                emit_qk_cast(pq2, 2, "act", part="q")
                pts_tiles[2] = pt_pool.tile(
                    [128, JP, 2, HPC, 512], fp8, tag="pts", name="pts2"
                )
                emit_round(2, 1, pts_tiles[2])
                # qg3 scores with PV(2)
                pq3 = alloc_qk("pq3")
                emit_q_proj(pq3, 3)
                emit_qk_cast(pq3, 3, "dve", part="q")
                pts_tiles[3] = pt_pool.tile(
                    [128, JP, 2, HPC, 512], fp8, tag="pts", name="pts3"
                )
                emit_round(3, 2, pts_tiles[3])
                # PV(3) pairs 4..7 trickle against the last exps
                for jp in range(4, JP):
                    emit_pv_pair(pus[3], jp, pts_tiles[3])
                emit_finalize(3, pus[3])
                emit_add_store(3, last=True)

    nc.finalize()
    return nc


def _get_nc(n_rows):
    if n_rows not in _CACHE:
        _CACHE[n_rows] = _build(n_rows)
    return _CACHE[n_rows]


def _run(tgt, memory, Wq, Wk, Wv, trace=False):
    global LAST_RESULTS
    from concourse.bass_utils import run_bass_kernel_spmd

    n_rows = tgt.shape[1]
    nc = _get_nc(n_rows)

    tgt = np.ascontiguousarray(tgt, dtype=np.float32)
    memory = np.ascontiguousarray(memory, dtype=np.float32)
    import ml_dtypes

    f8 = ml_dtypes.float8_e4m3
    tgt0t = np.ascontiguousarray(tgt[0].T).astype(f8)
    mem0t = np.ascontiguousarray(memory[0].T).astype(f8)

    in_maps = []
    for c in range(NCORES):
        sl = slice(c * CW, (c + 1) * CW)
        in_maps.append(
            {
                "tgt0t": tgt0t,
                "mem0t": mem0t,
                "wqt": np.ascontiguousarray(Wq[sl, :].T).astype(f8),
                "wkt": np.ascontiguousarray(Wk[sl, :].T).astype(f8),
                "wvt": np.ascontiguousarray(Wv[sl, :].T).astype(f8),
                "tgtc": np.ascontiguousarray(tgt[:, :, sl]).astype(np.float16),
            }
        )
    res = run_bass_kernel_spmd(nc, in_maps, list(range(NCORES)), trace=trace)
    LAST_RESULTS = res
    out = np.concatenate(
        [res.results[c]["outc"].astype(np.float32) for c in range(NCORES)], axis=2
    )
    return out


def kernel(tgt, memory, Wq, Wk, Wv):
    return _run(tgt, memory, Wq, Wk, Wv)


# revision 45
# speedup vs baseline: 1.1242x; 1.0010x over previous
import sys

sys.path.insert(0, "/opt/trn_rl_repo")

import numpy as np

NCORES = 8
B, FULL_N, D = 4, 2048, 1024
NH = 16
DK = 64  # head dim
HPC = NH // NCORES  # heads per core = 2
CW = HPC * DK  # output columns per core = 128
DC = D // 128  # D chunks = 8
VW = 80  # padded V width per head (64 dims + ones col + pad to %16)

# exp engine routing: score chunks with (2j+h) % 16 in ACT_PHASE go through
# ScalarE's exact Exp (fp8e4 out, scaled 2^-3); the rest through DVE's
# Schraudolph affine bit-hack (fp32 -> saturating uint8 == fp8e4m3 bits).
ACT_PHASE = frozenset({0, 2, 4, 6, 8, 10, 12, 14, 15})
# byte = psum_score * (8/ln2)/8 + (56 - 24 - sigma)  [fp8e4m3, y=exp(s)/8]
A_HACK = float(1.0 / np.log(2.0))  # 1.4427 (includes the 1/sqrt(dk)=1/8 fold)
B_HACK = 31.537  # 32 - 0.463 Schraudolph mantissa-balance

_CACHE = {}
LAST_RESULTS = None


def _build(n_rows):
    """SPMD Bass program for one core. Each core computes batch-0 attention
    for its 2 heads (the reference only uses att[0]) and adds it to its
    column slice of tgt for all batches.

    Structure (tuned against real-HW traces, not the cost model):
    - Q/K/V projections run fp8 DoubleRow on fp8-staged inputs; scores run
      fp16 (on this HW a matmul costs ~1 cycle per output column whatever
      the dtype/perf-mode, so fp8-DR scores buy nothing and slot-split
      projections are pure loss). V is projected directly transposed
      (memT as lhsT) so no PE transposes are needed for the PV operand.
    - Softmax exps (the 2nd bottleneck after the PE) split ScalarE (exact
      Exp, 9/16 chunks) and DVE (saturating affine-to-uint8 Schraudolph
      bit hack, 7/16); score PSUM tiles triple-buffer (sharing the pool
      with projection/finalize tiles) so neither engine idles on the
      score->exp->PV rotation.
    - PV runs fp8 DR with a ones column at position 64 of each 80-wide V
      block yielding softmax row sums for free; its accumulation pairs
      are spread one-per-odd-chunk through the score stream (clumped PV
      blocks starve the exp engines).
    - Each q-group's finalize/add/store is emitted mid-round so the
      in-order ACT/DVE queues never bury them behind the next group's
      exps; nothing but PV(3)+fin(3) remains after the last exp.
    - DMA: K-group feeds outrank Q feeds (scores need K first); the 2MB
      tgt/outc add-path loads ride the Pool queue behind filler memsets
      so they never steal inbound bandwidth from the critical loads.
    - Final adds ride GpSimd except the last q-group (split DVE/GpSimd
      to shorten the serial tail)."""
    import concourse.mybir as mybir
    import concourse.tile as tile
    from concourse import bacc
    from concourse.masks import make_identity

    fp32 = mybir.dt.float32
    fp16 = mybir.dt.float16
    bf16 = mybir.dt.bfloat16
    fp8 = mybir.dt.float8e4
    u8 = mybir.dt.uint8

    RT = n_rows // 128  # row tiles = 16
    G = n_rows // 512  # 512-row groups = 4
    QG = G
    KC = RT  # key chunks of 128
    JP = KC // 2  # key chunk pairs = 8

    nc = bacc.Bacc(None, target_bir_lowering=False)
    tgt0t = nc.declare_dram_parameter("tgt0t", [D, n_rows], fp8, isOutput=False)
    mem0t = nc.declare_dram_parameter("mem0t", [D, n_rows], fp8, isOutput=False)
    wqt = nc.declare_dram_parameter("wqt", [D, CW], fp8, isOutput=False)
    wkt = nc.declare_dram_parameter("wkt", [D, CW], fp8, isOutput=False)
    wvt = nc.declare_dram_parameter("wvt", [D, CW], fp8, isOutput=False)
    tgtc = nc.declare_dram_parameter("tgtc", [B, n_rows, CW], fp16, isOutput=False)
    outc = nc.declare_dram_parameter("outc", [B, n_rows, CW], fp16, isOutput=True)

    Exp = mybir.ActivationFunctionType.Exp
    DR = mybir.MatmulPerfMode.DoubleRow
    mult = mybir.AluOpType.mult
    add = mybir.AluOpType.add

    with tile.TileContext(nc) as tc:
        with (
            tc.tile_pool(name="const", bufs=1) as const,
            tc.tile_pool(name="persist", bufs=1) as persist,
        ):
            ident16 = const.tile([128, 128], fp16)
            make_identity(nc, ident16)
            bias_exp = const.tile([128, 1], fp32, tag="bias")
            nc.vector.memset(bias_exp, -3.0 * float(np.log(2.0)))
            wz = const.tile([128, 128], fp8, tag="wz")
            nc.vector.memset(wz, 0.0)

            # Q/K fp16 stores: partition = dim h*64+d, free = query/key
            QT = persist.tile([128, n_rows], fp16, tag="qt")
            KT = persist.tile([128, n_rows], fp16, tag="kt")
            # [keys, pair jp, pair half i, h*VW + dim]; ones at h*VW+64
            Vp = persist.tile([128, JP, 2, HPC * VW], fp8, tag="Vp")
            nc.gpsimd.memset(Vp, 0.0)
            Vp_h = Vp.rearrange("p jp i (h c) -> p jp i h c", h=HPC)
            nc.gpsimd.memset(Vp_h[:, :, :, :, DK : DK + 1], 1.0)

            att_sb = persist.tile([128, RT, CW], fp16, tag="att")
            tgtc_sb = persist.tile([128, B, RT, CW], fp16, tag="tgtc")

            with (
                tc.tile_pool(name="wst", bufs=1) as wst_pool,
                tc.tile_pool(name="memg", bufs=2) as mem_pool,
                tc.tile_pool(name="tgtg", bufs=2) as tgt_pool,
                tc.tile_pool(name="usb", bufs=2) as usb_pool,
                tc.tile_pool(name="small", bufs=4) as small_pool,
                tc.tile_pool(name="pt", bufs=2) as pt_pool,
                tc.tile_pool(name="ps_st", bufs=3, space="PSUM") as ps_st,
                tc.tile_pool(name="ps_u", bufs=2, space="PSUM") as ps_u,
            ):
                # --- initial DMAs ---
                # ACT queue: mem0 first (K0 gates the first scores), then the
                # Exp table, then mem2 (key groups keep landing while SP
                # drains tgt0/mem1). SP: weights, tgt0, mem1, tgt1, mem3, ...
                wt_q = wst_pool.tile([128, DC, CW], fp8, tag="wtq")
                wt_k = wst_pool.tile([128, DC, CW], fp8, tag="wtk")
                wt_v = wst_pool.tile([128, DC, CW], fp8, tag="wtv")
                WTs = {"q": wt_q, "k": wt_k, "v": wt_v}

                mem_tiles = {}
                tgt_tiles = {}

                def emit_mem_dma(g, eng):
                    t = mem_pool.tile([128, DC, 512], fp8, tag="memg", name=f"mem{g}")
                    eng.dma_start(
                        out=t,
                        in_=mem0t[:, g * 512 : (g + 1) * 512].rearrange(
                            "(c p) n -> p c n", p=128
                        ),
                    )
                    mem_tiles[g] = t

                def emit_tgt_dma(g, eng):
                    t = tgt_pool.tile([128, DC, 512], fp8, tag="tgtg", name=f"tgt{g}")
                    eng.dma_start(
                        out=t,
                        in_=tgt0t[:, g * 512 : (g + 1) * 512].rearrange(
                            "(c p) n -> p c n", p=128
                        ),
                    )
                    tgt_tiles[g] = t

                nc.sync.dma_start(
                    out=wt_q, in_=wqt[:, :].rearrange("(c p) q -> p c q", p=128)
                )
                emit_tgt_dma(0, nc.sync)
                nc.sync.dma_start(
                    out=wt_k, in_=wkt[:, :].rearrange("(c p) q -> p c q", p=128)
                )
                emit_mem_dma(0, nc.sync)
                # table preload for the exact-Exp chunks
                act_warm = const.tile([128, 1], fp32, tag="actw")
                nc.scalar.activation(out=act_warm, in_=bias_exp, func=Exp)
                emit_mem_dma(2, nc.scalar)
                emit_mem_dma(1, nc.sync)
                emit_mem_dma(3, nc.sync)
                nc.sync.dma_start(
                    out=wt_v, in_=wvt[:, :].rearrange("(c p) q -> p c q", p=128)
                )
                emit_tgt_dma(1, nc.sync)
                emit_tgt_dma(2, nc.sync)
                emit_tgt_dma(3, nc.sync)
                # tgtc loads ride the Pool queue BEHIND scratch-memset
                # fillers so their 2MB doesn't steal inbound DMA bandwidth
                # from the critical startup loads (Pool executes in order;
                # the adds that consume tgtc_sb only start much later).
                scratch = persist.tile([128, 4096], fp16, tag="scratch")
                for r in range(4):
                    nc.gpsimd.memset(scratch, 0.0)
                for b in range(B):
                    nc.gpsimd.dma_start(
                        out=tgtc_sb[:, b, 0:8, :],
                        in_=tgtc[b, 0:1024, :].rearrange("(t p) c -> p t c", p=128),
                    )
                for r in range(3):
                    nc.gpsimd.memset(scratch, 0.0)
                for b in range(B):
                    nc.gpsimd.dma_start(
                        out=tgtc_sb[:, b, 8:16, :],
                        in_=tgtc[b, 1024:2048, :].rearrange("(t p) c -> p t c", p=128),
                    )

                def alloc_qk(name):
                    # proj/vps/ta tiles live in the st pool (uniform shape;
                    # every consumer completes independently, so sharing is
                    # deadlock-free and buys a 3rd score buffer)
                    return ps_st.tile([128, 2, 512], fp32, tag="st", name=name)

                def emit_k_proj(pk, g):
                    src = mem_tiles[g]
                    for dp in range(DC // 2):
                        nc.tensor.matmul(
                            pk[:, 0, :],
                            WTs["k"][:, 2 * dp : 2 * dp + 2, :],
                            src[:, 2 * dp : 2 * dp + 2, :],
                            start=(dp == 0), stop=(dp == DC // 2 - 1),
                            perf_mode=DR,
                        )

                def emit_q_proj(pk, g):
                    src = tgt_tiles[g]
                    for dp in range(DC // 2):
                        nc.tensor.matmul(
                            pk[:, 0, :],
                            WTs["q"][:, 2 * dp : 2 * dp + 2, :],
                            src[:, 2 * dp : 2 * dp + 2, :],
                            start=(dp == 0), stop=(dp == DC // 2 - 1),
                            perf_mode=DR,
                        )

                def emit_qk_cast(pk, g, eng, part):
                    dst = (KT if part == "k" else QT)[:, g * 512 : (g + 1) * 512]
                    src = pk[:, 0, :]
                    if eng == "act":
                        nc.scalar.copy(out=dst, in_=src)
                    else:
                        nc.vector.tensor_copy(out=dst, in_=src)

                def alloc_u(name):
                    return ps_u.tile([128, 512], fp32, tag="u", name=name)

                def emit_vt(g):
                    """V^T for group g: [128 keys, 4 chunks, 128 dims] via
                    DR matmuls with memT as lhsT (no transposes needed).
                    Lives in the u pool so its cast never blocks the
                    exp-paced score-tile rotation."""
                    vt = alloc_u(f"vps{g}")
                    vps = vt.rearrange("p (t c) -> p t c", t=4)
                    src = mem_tiles[g]
                    for t in range(4):
                        for dp in range(DC // 2):
                            nc.tensor.matmul(
                                vps[:, t, :],
                                src[:, 2 * dp : 2 * dp + 2, t * 128 : (t + 1) * 128],
                                WTs["v"][:, 2 * dp : 2 * dp + 2, :],
                                start=(dp == 0), stop=(dp == DC // 2 - 1),
                                perf_mode=DR,
                            )
                    return vps

                def emit_v_cast(g, vps, eng):
                    # [128, (jp2 i2), (h2, 64)] -> Vp[:, 2g:2g+2, :, :, 0:64]
                    src = vps.rearrange("p (jp i) (h c) -> p jp i h c", jp=2, h=HPC)
                    dst = Vp_h[:, 2 * g : 2 * g + 2, :, :, 0:DK]
                    if eng == "act":
                        nc.scalar.copy(out=dst, in_=src)
                    else:
                        nc.vector.tensor_copy(out=dst, in_=src)

                def emit_score_chunk(qg, j, pts):
                    """Score matmuls (one per head; each out [128,512] fits a
                    single PSUM bank) + one both-heads exp for key chunk j."""
                    st = ps_st.tile(
                        [128, HPC, 512], fp32, tag="st", name=f"st{qg}_{j}"
                    )
                    for h in range(HPC):
                        hs = h * DK
                        nc.tensor.matmul(
                            st[:, h, :],
                            KT[hs : hs + DK, j * 128 : (j + 1) * 128],
                            QT[hs : hs + DK, qg * 512 : (qg + 1) * 512],
                            start=True, stop=True,
                        )
                    jp, i = j // 2, j % 2
                    dst = pts[:, jp, i, :, :]  # [128, h, 512] fp8
                    if j in ACT_PHASE:
                        nc.scalar.activation(
                            out=dst, in_=st, func=Exp, scale=0.125, bias=bias_exp
                        )
                    else:
                        nc.vector.tensor_scalar(
                            out=dst.bitcast(u8), in0=st,
                            scalar1=A_HACK, scalar2=B_HACK,
                            op0=mult, op1=add,
                        )

                def emit_pv_pair(pu_list, jp, pts):
                    for h in range(HPC):
                        nc.tensor.matmul(
                            pu_list[h][0:VW, :],
                            Vp[:, jp, :, h * VW : (h + 1) * VW],
                            pts[:, jp, :, h, :],
                            start=(jp == 0), stop=(jp == JP - 1),
                            perf_mode=DR,
                        )

                def emit_pv(qg, pu_list, pts):
                    for jp in range(JP):
                        emit_pv_pair(pu_list, jp, pts)

                def emit_finalize(qg, pu_list):
                    """pu [80,512] PSUM -> fp16 SBUF casts (h0 DVE, h1 ACT),
                    8 PE transposes into a recycled qk-pool tile (bitcast to
                    fp16), one batched reciprocal of the ones-column sums,
                    then 2x-mode scales writing att_sb."""
                    pu_sbs = []
                    for h in range(HPC):
                        pu_sb = usb_pool.tile([VW, 512], fp16, tag="usb")
                        if h == 1:
                            nc.scalar.copy(out=pu_sb, in_=pu_list[h][0:VW, :])
                        else:
                            nc.vector.tensor_copy(out=pu_sb, in_=pu_list[h][0:VW, :])
                        pu_sbs.append(pu_sb)
                    ta_raw = alloc_qk(f"ta{qg}")
                    # fp16 view: [128, 2h, 4s, 256] (only 0:80 of each used)
                    ta = ta_raw.bitcast(fp16).rearrange(
                        "p h (s c) -> p h s c", s=4
                    )
                    for h in range(HPC):
                        for s in range(4):
                            nc.tensor.transpose(
                                ta[:, h, s, 0:VW],
                                pu_sbs[h][:, s * 128 : (s + 1) * 128],
                                ident16[0:VW, 0:VW],
                            )
                    rec = small_pool.tile([128, 2, 4, 1], fp32, tag="rec")
                    nc.vector.reciprocal(rec, ta[:, :, :, DK : DK + 1])
                    for h in range(HPC):
                        nc.vector.tensor_tensor(
                            out=att_sb[:, qg * 4 : (qg + 1) * 4, h * DK : (h + 1) * DK],
                            in0=ta[:, h, :, 0:DK],
                            in1=rec[:, h, :, :].to_broadcast((128, 4, DK)),
                            op=mult,
                        )

                def emit_add_store(qg, last):
                    qsl = slice(qg * 512, (qg + 1) * 512)
                    for b in range(B):
                        eng = nc.vector if (last and b % 2 == 0) else nc.gpsimd
                        eng.tensor_add(
                            out=tgtc_sb[:, b, qg * 4 : (qg + 1) * 4, :],
                            in0=tgtc_sb[:, b, qg * 4 : (qg + 1) * 4, :],
                            in1=att_sb[:, qg * 4 : (qg + 1) * 4, :],
                        )
                        eng2 = nc.scalar if (last and b >= 2) else nc.sync
                        eng2.dma_start(
                            out=outc[b, qsl, :].rearrange("(t p) c -> p t c", p=128),
                            in_=tgtc_sb[:, b, qg * 4 : (qg + 1) * 4, :],
                        )

                # ---- Phase A: projections, V, qg0+qg1 scores ----
                pts_tiles = {}
                pus = {}
                pts_tiles[0] = pt_pool.tile(
                    [128, JP, 2, HPC, 512], fp8, tag="pts", name="pts0"
                )
                pts_tiles[1] = pt_pool.tile(
                    [128, JP, 2, HPC, 512], fp8, tag="pts", name="pts1"
                )

                pq0 = alloc_qk("pq0")
                for _ in range(20):
                    nc.tensor.matmul(
                        pq0[:, 0, 0:128], wz, wz,
                        start=True, stop=True, skip_group_check=True,
                    )
                emit_q_proj(pq0, 0)
                emit_qk_cast(pq0, 0, "dve", part="q")
                pk0 = alloc_qk("pk0")
                emit_k_proj(pk0, 0)
                emit_qk_cast(pk0, 0, "act", part="k")
                for j in range(0, 4):
                    emit_score_chunk(0, j, pts_tiles[0])
                vps0 = emit_vt(0)
                emit_v_cast(0, vps0, "dve")
                # g2 landed early on the ACT queue
                pk2 = alloc_qk("pk2")
                emit_k_proj(pk2, 2)
                emit_qk_cast(pk2, 2, "act", part="k")
                for j in range(8, 12):
                    emit_score_chunk(0, j, pts_tiles[0])
                vps2 = emit_vt(2)
                emit_v_cast(2, vps2, "act")
                pq1 = alloc_qk("pq1")
                emit_q_proj(pq1, 1)
                emit_qk_cast(pq1, 1, "dve", part="q")
                pk1 = alloc_qk("pk1")
                emit_k_proj(pk1, 1)
                emit_qk_cast(pk1, 1, "act", part="k")
                for j in range(4, 8):
                    emit_score_chunk(0, j, pts_tiles[0])
                vps1 = emit_vt(1)
                emit_v_cast(1, vps1, "dve")
                pk3 = alloc_qk("pk3")
                emit_k_proj(pk3, 3)
                emit_qk_cast(pk3, 3, "dve", part="k")
                for j in range(12, 16):
                    emit_score_chunk(0, j, pts_tiles[0])
                vps3 = emit_vt(3)
                emit_v_cast(3, vps3, "act")
                pus[0] = [alloc_u(f"u0_{h}") for h in range(HPC)]
                for jp in range(4):
                    emit_pv_pair(pus[0], jp, pts_tiles[0])

                # ---- Phase B ----
                # Strict qg-major chunk order; each PV chain's pairs are
                # interleaved two-at-a-time into the first half of the NEXT
                # q-group's score stream (deps are ~16 chunks stale), and
                # fin/add/store fire immediately after the chain closes so
                # nothing cascades into the tail.
                def emit_round(g_sc, qg_pv, pts_sc):
                    # first half: scores(g_sc, 0..7) with PV(qg_pv) pairs
                    # 4..7 (one per odd chunk); fin/add/store(qg_pv); second
                    # half: scores 8..15 with PV(g_sc-1) pairs 0..3 --
                    # spreading PV evenly keeps the PE from starving the
                    # exp stream in bursts.
                    for j in range(0, 8):
                        emit_score_chunk(g_sc, j, pts_sc)
                        if j % 2 == 1:
                            emit_pv_pair(pus[qg_pv], 4 + j // 2, pts_tiles[qg_pv])
                    emit_finalize(qg_pv, pus[qg_pv])
                    emit_add_store(qg_pv, last=(qg_pv == QG - 1))
                    qg_n = g_sc
                    pus[qg_n] = [alloc_u(f"u{qg_n}_{h}") for h in range(HPC)]
                    for j in range(8, 16):
                        emit_score_chunk(g_sc, j, pts_sc)
                        if j % 2 == 1:
                            emit_pv_pair(pus[qg_n], (j - 9) // 2, pts_tiles[qg_n])

                # qg1 scores with PV(0) interleaved
                emit_round(1, 0, pts_tiles[1])
                # qg2 scores with PV(1)
                pq2 = alloc_qk("pq2")
                emit_q_proj(pq2, 2)
                emit_qk_cast(pq2, 2, "act", part="q")
                pts_tiles[2] = pt_pool.tile(
                    [128, JP, 2, HPC, 512], fp8, tag="pts", name="pts2"
                )
                emit_round(2, 1, pts_tiles[2])
                # qg3 scores with PV(2)
                pq3 = alloc_qk("pq3")
                emit_q_proj(pq3, 3)
                emit_qk_cast(pq3, 3, "dve", part="q")
                pts_tiles[3] = pt_pool.tile(
                    [128, JP, 2, HPC, 512], fp8, tag="pts", name="pts3"
                )
                emit_round(3, 2, pts_tiles[3])
                # PV(3) pairs 4..7 trickle against the last exps
                for jp in range(4, JP):
                    emit_pv_pair(pus[3], jp, pts_tiles[3])
                emit_finalize(3, pus[3])
                emit_add_store(3, last=True)

    nc.finalize()
    return nc


def _get_nc(n_rows):
    if n_rows not in _CACHE:
        _CACHE[n_rows] = _build(n_rows)
    return _CACHE[n_rows]


def _run(tgt, memory, Wq, Wk, Wv, trace=False):
    global LAST_RESULTS
    from concourse.bass_utils import run_bass_kernel_spmd

    n_rows = tgt.shape[1]
    nc = _get_nc(n_rows)

    tgt = np.ascontiguousarray(tgt, dtype=np.float32)
    memory = np.ascontiguousarray(memory, dtype=np.float32)
    import ml_dtypes

    f8 = ml_dtypes.float8_e4m3
    tgt0t = np.ascontiguousarray(tgt[0].T).astype(f8)
    mem0t = np.ascontiguousarray(memory[0].T).astype(f8)

    in_maps = []
    for c in range(NCORES):
        sl = slice(c * CW, (c + 1) * CW)
        in_maps.append(
            {
                "tgt0t": tgt0t,
                "mem0t": mem0t,
                "wqt": np.ascontiguousarray(Wq[sl, :].T).astype(f8),
                "wkt": np.ascontiguousarray(Wk[sl, :].T).astype(f8),
                "wvt": np.ascontiguousarray(Wv[sl, :].T).astype(f8),
                "tgtc": np.ascontiguousarray(tgt[:, :, sl]).astype(np.float16),
            }
        )
    res = run_bass_kernel_spmd(nc, in_maps, list(range(NCORES)), trace=trace)
    LAST_RESULTS = res
    out = np.concatenate(
        [res.results[c]["outc"].astype(np.float32) for c in range(NCORES)], axis=2
    )
    return out


def kernel(tgt, memory, Wq, Wk, Wv):
    return _run(tgt, memory, Wq, Wk, Wv)
